# revision 1
# baseline (speedup 1.0000x reference)
"""Bidirectional Mamba block on 8 Trainium2 NeuronCores (Bass/Tile).

Data-parallel over batch: B=16 -> 2 per core; weights replicated; host gathers.
Per-core layout is feature-major ([feature_partitions, tokens]) with tokens =
batch-major concatenation of the 2 local sequences (t = b*512 + l).

Engines:
  PE   - all projections (weights stationary as lhsT), depthwise causal conv as
         4 accumulating diag-matmuls over shifted views, partition-broadcast of
         per-token B/C rows via one-hot selector matmuls.
  ACT  - exp/ln resident table only: softplus = ln(exp(.)+1), silu via exp,
         rsqrt = exp(-0.5*ln(.)); dA_n = exp(delta * A[:,n]) with per-partition
         scale; fused PSUM->SBUF copies.
  DVE  - selective scan via tensor_tensor_scan (fp32 internal state); the
         backward layer feeds the scan with reversed access patterns.
"""

import numpy as np

# ---- problem constants (hardcoded per contract) ----
B, L, DM = 16, 512, 256
DI, N, R, KC = 512, 16, 16, 4
NCORES = 8
BL = B // NCORES          # local batch
TOK = BL * L              # 1024 tokens per core
DT_TILES = DI // 128      # 4
MT = DM // 128            # 2
F32_np = np.float32

# ---- dtype knobs for the scan path ----
import ml_dtypes
BF16_np = ml_dtypes.bfloat16

CFG = dict(
    DA="bf16",     # dA (scan decay operand)
    DELTA="bf16",  # delta resident
    W="bf16",      # w = delta*xs (scan drive factor)
    H="bf16",      # scan output h
    REP="bf16",    # B_rep / C_rep broadcast tiles
    P="bf16",      # products h*C
    YACC="bf16",   # y accumulator (only the non-PE d-tile)
    SZ="bf16",     # silu(z) gate
    XS="bf16",     # conv-silu output / gate buffer
    SILU="exp",    # "sigmoid" table or "exp"+reciprocal
    PROBE="",      # timing probes: shrink a stage's work (breaks numerics)
    ADDS="pe",     # y_acc adds: "pe" (psum identity-matmul), "dve", "pool"
)

_BUILD_CACHE = {}


# ======================================================================
# host-side weight preparation
# ======================================================================

def _prep_layer_weights(inw, convw, convb, xprojw, dtw, dtb, Alog, Dp, outw, normw):
    """Fold/reshape one mamba layer's weights into device layouts."""
    out = {}
    # in_proj with rmsnorm weight folded into rows: [128, 2, 1024]
    w = (np.asarray(normw)[:, None] * np.asarray(inw)).astype(F32_np)
    out["inw"] = np.ascontiguousarray(w.reshape(2, 128, 2 * DI).transpose(1, 0, 2)).astype(BF16_np)
    # conv diag matrices: [128, 16(dt*4+k), 128]
    cd = np.zeros((128, DT_TILES * KC, 128), F32_np)
    cw = np.asarray(convw).astype(F32_np)  # (KC, 1, DI)
    for dt in range(DT_TILES):
        for k in range(KC):
            idx = np.arange(128)
            cd[idx, dt * KC + k, idx] = cw[k, 0, dt * 128 + idx]
    out["convd"] = np.ascontiguousarray(cd).astype(BF16_np)
    out["convbn"] = np.ascontiguousarray(
        (-np.asarray(convb).astype(F32_np)).reshape(DT_TILES, 128, 1).transpose(1, 0, 2))
    out["convb"] = np.ascontiguousarray(
        np.asarray(convb).astype(F32_np).reshape(DT_TILES, 128, 1).transpose(1, 0, 2))
    # xproj padded so delta_raw/B/C land at partitions 0/32/64: [128, 4, 96]
    xp = np.zeros((DI, 96), F32_np)
    xpw = np.asarray(xprojw).astype(F32_np)
    xp[:, 0:R] = xpw[:, 0:R]
    xp[:, 32:32 + N] = xpw[:, R:R + N]
    xp[:, 64:64 + N] = xpw[:, R + N:R + 2 * N]
    out["xpw"] = np.ascontiguousarray(xp.reshape(DT_TILES, 128, 96).transpose(1, 0, 2)).astype(BF16_np)
    out["dtw"] = np.ascontiguousarray(np.asarray(dtw).astype(F32_np)).astype(BF16_np)          # (16, 512)
    out["dtb"] = np.ascontiguousarray(
        np.asarray(dtb).astype(F32_np).reshape(DT_TILES, 128, 1).transpose(1, 0, 2))
    A = (-np.exp(np.asarray(Alog).astype(np.float64))).astype(F32_np)          # (512, 16)
    out["A"] = np.ascontiguousarray(A.reshape(DT_TILES, 128, N).transpose(1, 0, 2))
    out["Dp"] = np.ascontiguousarray(
        np.asarray(Dp).astype(F32_np).reshape(DT_TILES, 128, 1).transpose(1, 0, 2))
    out["outw"] = np.ascontiguousarray(
        np.asarray(outw).astype(F32_np).reshape(DT_TILES, 128, DM).transpose(1, 0, 2)).astype(BF16_np)
    return out


def _prep_shared_weights(proj_w, proj_b, ln_g, ln_b):
    out = {}
    out["projw"] = np.ascontiguousarray(
        np.asarray(proj_w).astype(F32_np).reshape(4, 128, DM).transpose(1, 0, 2)).astype(BF16_np)
    out["projb"] = np.ascontiguousarray(
        np.asarray(proj_b).astype(F32_np).reshape(MT, 128, 1).transpose(1, 0, 2))
    out["lng"] = np.ascontiguousarray(
        np.asarray(ln_g).astype(F32_np).reshape(MT, 128, 1).transpose(1, 0, 2))
    out["lnb"] = np.ascontiguousarray(
        np.asarray(ln_b).astype(F32_np).reshape(MT, 128, 1).transpose(1, 0, 2))
    return out


# ======================================================================
# device program
# ======================================================================

def _build(loop_k=1, cfg=None, variant="full"):
    cfg = dict(CFG if cfg is None else cfg)
    key = (loop_k, variant, tuple(sorted(cfg.items())))
    if key in _BUILD_CACHE:
        return _BUILD_CACHE[key]

    import concourse.bacc as bacc
    import concourse.mybir as mybir
    import concourse.tile as tile

    F32 = mybir.dt.float32
    BF16 = mybir.dt.bfloat16
    AF = mybir.ActivationFunctionType
    ALU = mybir.AluOpType
    AX = mybir.AxisListType

    def dt_of(kname):
        return F32 if cfg[kname] == "f32" else BF16

    nc = bacc.Bacc("TRN2", target_bir_lowering=False, debug=False)

    def din(name, shape, dt=None):
        return nc.dram_tensor(name, list(shape), dt or F32, kind="ExternalInput").ap()

    # --- DRAM I/O ---
    xT_d = din("xT", (DM, TOK))
    lw_d = {}
    for s in ("f", "b"):
        lw_d[s] = {
            "inw": din(f"{s}_inw", (128, 2, 2 * DI), BF16),
            "convd": din(f"{s}_convd", (128, DT_TILES * KC, 128), BF16),
            "convbn": din(f"{s}_convbn", (128, DT_TILES, 1)),
            "convb": din(f"{s}_convb", (128, DT_TILES, 1)),
            "xpw": din(f"{s}_xpw", (128, DT_TILES, 96), BF16),
            "dtw": din(f"{s}_dtw", (16, DI), BF16),
            "dtb": din(f"{s}_dtb", (128, DT_TILES, 1)),
            "A": din(f"{s}_A", (128, DT_TILES, N)),
            "Dp": din(f"{s}_Dp", (128, DT_TILES, 1)),
            "outw": din(f"{s}_outw", (128, DT_TILES, DM), BF16),
        }
    projw_d = din("projw", (128, 4, DM), BF16)
    projb_d = din("projb", (128, MT, 1))
    lng_d = din("lng", (128, MT, 1))
    lnb_d = din("lnb", (128, MT, 1))
    outT_d = nc.dram_tensor("outT", [DM, TOK], F32, kind="ExternalOutput").ap()

    PAD = KC - 1  # 3
    CONVW = 2 * PAD + L  # padded per-batch row length 518

    with tile.TileContext(nc) as tc:
        from contextlib import ExitStack
        with ExitStack() as ctx:
            wpool = ctx.enter_context(tc.tile_pool(name="wpool", bufs=1))
            pers = ctx.enter_context(tc.tile_pool(name="pers", bufs=1))
            work = ctx.enter_context(tc.tile_pool(name="work", bufs=2))
            rep = ctx.enter_context(tc.tile_pool(name="rep", bufs=2 if cfg["ADDS"] == "pe" else 1))
            scanw = ctx.enter_context(tc.tile_pool(name="scanw", bufs=2))
            sbufs3 = 3 if cfg.get("LOOKAHEAD") == "3" else None

            def body():
                # ---- load shared weights ----
                projw_t = wpool.tile([128, 4, DM], BF16, tag="projw", name="projw")
                nc.sync.dma_start(projw_t[:], projw_d[:])
                projb_t = wpool.tile([128, MT, 1], F32, tag="projb", name="projb")
                nc.sync.dma_start(projb_t[:], projb_d[:])
                lng_t = wpool.tile([128, MT, 1], F32, tag="lng", name="lng")
                nc.sync.dma_start(lng_t[:], lng_d[:])
                lnb_t = wpool.tile([128, MT, 1], F32, tag="lnb", name="lnb")
                nc.sync.dma_start(lnb_t[:], lnb_d[:])

                xT = []
                for m in range(MT):
                    t = pers.tile([128, TOK], F32, tag=f"xT{m}", name=f"xT{m}")
                    nc.sync.dma_start(t[:], xT_d[m * 128:(m + 1) * 128, :])
                    xT.append(t)

                # ---- shared RMSNorm: xn = x * rsqrt(mean(x^2) + eps) ----
                xn = []
                with tc.tile_pool(name="prms", bufs=1, space="PSUM") as prms:
                    ones_col = wpool.tile([128, 1], F32, tag="ones_col", name="ones_col")
                    nc.vector.memset(ones_col[:], 1.0)
                    ss_ps = prms.tile([1, TOK], F32, tag="ss", name="ss")
                    for fh in range(2):
                        fs = slice(fh * 512, (fh + 1) * 512)
                        for m in range(MT):
                            sq = work.tile([128, 512], F32, tag="sqtmp", name="rms_sq")
                            nc.scalar.square(sq[:], xT[m][:, fs])
                            nc.tensor.matmul(ss_ps[:, fs],ones_col[:],sq[:],
                                             start=(m == 0), stop=(m == MT - 1))
                    # rs = exp(-0.5 * ln(ss/DM + eps))
                    eps1 = wpool.tile([1, 1], F32, tag="eps1", name="eps1")
                    nc.vector.memset(eps1[:], 1e-5)
                    rs_row = work.tile([1, TOK], F32, tag="rowtmp", name="rs_row")
                    nc.scalar.activation(rs_row[:], ss_ps[:], AF.Ln,
                                         scale=1.0 / DM, bias=eps1[:, 0:1])
                    nc.scalar.activation(rs_row[:], rs_row[:], AF.Exp, scale=-0.5)
                    ones1 = wpool.tile([1, 128], F32, tag="ones1", name="ones1")
                    nc.vector.memset(ones1[:], 1.0)
                    rs_ps = prms.tile([128, TOK], F32, tag="rs_rep", name="rs_rep")
                    for fh in range(2):
                        fs = slice(fh * 512, (fh + 1) * 512)
                        nc.tensor.matmul(rs_ps[:, fs],ones1[:],rs_row[:, fs],
                                         start=True, stop=True)
                    for m in range(MT):
                        t = pers.tile([128, TOK], BF16, tag=f"xn{m}", name=f"xn{m}")
                        nc.vector.tensor_mul(t[:], xT[m][:], rs_ps[:])
                        xn.append(t)

                # ---- one mamba layer ----
                def mamba_layer(s, reverse):
                    W = lw_d[s]
                    inw_t = wpool.tile([128, 2, 2 * DI], BF16, tag="inw", name="inw")
                    nc.sync.dma_start(inw_t[:], W["inw"][:])
                    convd_t = wpool.tile([128, DT_TILES * KC, 128], BF16, tag="convd", name="convd")
                    nc.sync.dma_start(convd_t[:], W["convd"][:])
                    convbn_t = wpool.tile([128, DT_TILES, 1], F32, tag="convbn", name="convbn")
                    nc.sync.dma_start(convbn_t[:], W["convbn"][:])
                    convb_t = wpool.tile([128, DT_TILES, 1], F32, tag="convb", name="convb")
                    nc.sync.dma_start(convb_t[:], W["convb"][:])
                    xpw_t = wpool.tile([128, DT_TILES, 96], BF16, tag="xpw", name="xpw")
                    nc.sync.dma_start(xpw_t[:], W["xpw"][:])
                    dtw_t = wpool.tile([16, DI], BF16, tag="dtw", name="dtw")
                    nc.sync.dma_start(dtw_t[:], W["dtw"][:])
                    dtb_t = wpool.tile([128, DT_TILES, 1], F32, tag="dtb", name="dtb")
                    nc.sync.dma_start(dtb_t[:], W["dtb"][:])
                    A_t = wpool.tile([128, DT_TILES, N], F32, tag="A", name="A")
                    nc.sync.dma_start(A_t[:], W["A"][:])
                    Dp_t = wpool.tile([128, DT_TILES, 1], F32, tag="Dp", name="Dp")
                    nc.sync.dma_start(Dp_t[:], W["Dp"][:])
                    outw_t = wpool.tile([128, DT_TILES, DM], BF16, tag="outw", name="outw")
                    nc.sync.dma_start(outw_t[:], W["outw"][:])

                    xmpad = []
                    sz = []
                    xs = []
                    for dt in range(DT_TILES):
                        t = pers.tile([128, BL, CONVW], BF16, tag=f"xmpad{dt}", name=f"xmpad{dt}")
                        nc.vector.memset(t[:, :, 0:PAD], 0.0)
                        nc.vector.memset(t[:, :, PAD + L:CONVW], 0.0)
                        xmpad.append(t)
                        sz.append(pers.tile([128, TOK], dt_of("SZ"), tag=f"sz{dt}", name=f"sz{dt}"))
                        xs.append(pers.tile([128, TOK], dt_of("XS"), tag=f"xs{dt}", name=f"xs{dt}"))

                    # ---- in_proj ----
                    with tc.tile_pool(name="pp", bufs=4, space="PSUM") as pp:
                        for m in range(8):
                            for fh in range(2):
                                fs = slice(fh * 512, (fh + 1) * 512)
                                ps = pp.tile([128, 512], F32, tag="pp", name="pp")
                                for ks in range(2):
                                    nc.tensor.matmul(
                                        ps[:],inw_t[:, ks, m * 128:(m + 1) * 128],xn[ks][:, fs], start=(ks == 0), stop=(ks == 1))
                                if m < 4:
                                    # xm -> padded conv buffer (fh == local batch idx)
                                    nc.vector.tensor_copy(xmpad[m][:, fh, PAD:PAD + L], ps[:])
                                else:
                                    zdt = m - 4
                                    e = work.tile([128, 512], F32, tag="zetag", name="ze")
                                    if cfg["SILU"] == "sigmoid":
                                        nc.scalar.activation(e[:], ps[:], AF.Sigmoid)
                                    else:
                                        nc.scalar.activation(e[:], ps[:], AF.Exp, scale=-1.0)
                                        nc.vector.tensor_scalar(e[:], e[:], 1.0, None, ALU.add)
                                        nc.vector.reciprocal(e[:], e[:])
                                    nc.vector.tensor_mul(sz[zdt][:, fs], e[:], ps[:])

                        # ---- depthwise causal conv + silu ----
                        for dt in range(DT_TILES):
                            for b in range(BL):
                                ps = pp.tile([128, 512], F32, tag="pp", name="pp")
                                for k in range(KC):
                                    off = k if not reverse else (2 * PAD - k)
                                    nc.tensor.matmul(
                                        ps[:],convd_t[:, dt * KC + k, :],xmpad[dt][:, b, off:off + L],
                                        start=(k == 0), stop=(k == KC - 1))
                                bs = slice(b * L, (b + 1) * L)
                                e = work.tile([128, 512], F32, tag="cetag", name="ce")
                                if cfg["SILU"] == "sigmoid":
                                    nc.scalar.activation(e[:], ps[:], AF.Sigmoid,
                                                         bias=convb_t[:, dt, 0:1])
                                else:
                                    nc.scalar.activation(e[:], ps[:], AF.Exp, scale=-1.0,
                                                         bias=convbn_t[:, dt, 0:1])
                                    nc.vector.tensor_scalar(e[:], e[:], 1.0, None, ALU.add)
                                    nc.vector.reciprocal(e[:], e[:])
                                # xs = (conv + bias) * sigmoid  (silu)
                                nc.vector.scalar_tensor_tensor(
                                    xs[dt][:, bs], ps[:], convb_t[:, dt, 0:1], e[:],
                                    ALU.add, ALU.mult)

                    if cfg["PROBE"] == "stop_conv":
                        return [xs[0], xs[1]]
                    # ---- xproj -> delta_raw / Brows / Crows ----
                    dbc = pers.tile([16, 2, TOK], BF16, tag="dbc", name="dbc")
                    draw_t = work.tile([16, TOK], BF16, tag="draw", name="draw_t")
                    draw = draw_t[:, :]
                    Brows = dbc[:, 0, :]
                    Crows = dbc[:, 1, :]
                    with tc.tile_pool(name="pxp", bufs=1, space="PSUM") as pxp:
                        psx = pxp.tile([96, TOK], F32, tag="pxp", name="pxp")
                        for fh in range(2):
                            fs = slice(fh * 512, (fh + 1) * 512)
                            for ks in range(DT_TILES):
                                nc.tensor.matmul(psx[:, fs],xpw_t[:, ks, :],xs[ks][:, fs],
                                                 start=(ks == 0), stop=(ks == DT_TILES - 1))
                        nc.scalar.copy(draw, psx[0:16, :])
                        nc.scalar.copy(Brows, psx[32:48, :])
                        nc.scalar.copy(Crows, psx[64:80, :])

                    # ---- dt_proj + softplus -> delta; w = delta * xs ----
                    delta = []
                    w_t = []
                    with tc.tile_pool(name="pdt", bufs=3, space="PSUM") as pdt:
                        for dt in range(DT_TILES):
                            dl = pers.tile([128, TOK], dt_of("DELTA"), tag=f"delta{dt}", name=f"delta{dt}")
                            for fh in range(2):
                                fs = slice(fh * 512, (fh + 1) * 512)
                                ps = pdt.tile([128, 512], F32, tag="pdt", name="pdt")
                                nc.tensor.matmul(ps[:],dtw_t[:, dt * 128:(dt + 1) * 128],draw[:, fs], start=True, stop=True)
                                e = work.tile([128, 512], F32, tag="detag", name="de")
                                nc.scalar.activation(e[:], ps[:], AF.Exp,
                                                     bias=dtb_t[:, dt, 0:1])
                                nc.scalar.activation(dl[:, fs], e[:], AF.Ln, bias=1.0)
                            delta.append(dl)
                            wt = pers.tile([128, TOK], dt_of("W"), tag=f"w{dt}", name=f"w{dt}")
                            nc.vector.tensor_mul(wt[:], dl[:], xs[dt][:])
                            w_t.append(wt)

                    if cfg["PROBE"] == "stop_dt":
                        return [xs[0], xs[1]]
                    # ---- selective scan ----
                    use_pe = cfg["ADDS"] == "pe"
                    N_PE_DT = 3 if use_pe else 0   # d-tiles accumulated on PE/PSUM
                    if use_pe:
                        idn = wpool.tile([128, 128], BF16, tag="idn", name="idn")
                        from concourse.masks import make_identity
                        make_identity(nc, idn[:])
                    y_acc = [None if dt < N_PE_DT else
                             pers.tile([128, TOK], dt_of("YACC"), tag=f"yacc{dt}", name=f"yacc{dt}")
                             for dt in range(DT_TILES)]
                    pe_pool_ctx = tc.tile_pool(name="pyac", bufs=1, space="PSUM") if use_pe else None
                    if pe_pool_ctx is not None:
                        pyac = pe_pool_ctx.__enter__()
                        y_ps = [pyac.tile([128, TOK], F32, tag=f"yps{dt}", name=f"yps{dt}")
                                for dt in range(N_PE_DT)]
                    with tc.tile_pool(name="prep", bufs=1, space="PSUM") as prep:
                        for n in range(N):
                            seln = work.tile([16, 128], BF16, tag="seln", name="seln")
                            nc.gpsimd.memset(seln[:], 0.0)
                            nc.gpsimd.affine_select(
                                out=seln[:], in_=seln[:], compare_op=ALU.not_equal,
                                fill=1.0, base=-n, pattern=[[0, 128]],
                                channel_multiplier=1)
                            B_rep = rep.tile([128, TOK], dt_of("REP"), tag="B_rep", name="B_rep")
                            C_rep = rep.tile([128, TOK], dt_of("REP"), tag="C_rep", name="C_rep")
                            for fh in range(2):
                                fs = slice(fh * 512, (fh + 1) * 512)
                                psB = prep.tile([128, 512], F32, tag="psB", name="psB", bufs=1)
                                nc.tensor.matmul(psB[:], seln[:], Brows[:, fs], start=True, stop=True)
                                nc.scalar.copy(B_rep[:, fs], psB[:])
                                psC = prep.tile([128, 512], F32, tag="psC", name="psC", bufs=1)
                                nc.tensor.matmul(psC[:], seln[:], Crows[:, fs], start=True, stop=True)
                                nc.scalar.copy(C_rep[:, fs], psC[:])

                            for dt in range(DT_TILES):
                                dA = scanw.tile([128, TOK], dt_of("DA"), tag="dA", name="dA")
                                _dsl = slice(0, 64) if cfg["PROBE"] in ("dA", "acts") else slice(0, TOK)
                                nc.scalar.activation(dA[:, _dsl], delta[dt][:, _dsl], AF.Exp,
                                                     scale=A_t[:, dt, n:n + 1])
                                bx = scanw.tile([128, TOK], dt_of("W"), tag="bx", name="bx", bufs=sbufs3)
                                _bsl = slice(0, 64) if cfg["PROBE"] == "tt" else slice(0, TOK)
                                nc.vector.tensor_mul(bx[:, _bsl], w_t[dt][:, _bsl], B_rep[:, _bsl])
                                h = scanw.tile([128, TOK], dt_of("H"), tag="h", name="h", bufs=sbufs3)
                                if variant == "noscan":
                                    nc.vector.tensor_mul(h[:], dA[:], bx[:])
                                else:
                                    for b in range(BL):
                                        bs = slice(b * L, (b + 1) * L)
                                        if not reverse:
                                            nc.vector.tensor_tensor_scan(
                                                h[:, bs], dA[:, bs], bx[:, bs], 0.0,
                                                ALU.mult, ALU.add)
                                        else:
                                            nc.vector.tensor_tensor_scan(
                                                h[:, bs], dA[:, bs][:, ::-1],
                                                bx[:, bs][:, ::-1], 0.0,
                                                ALU.mult, ALU.add)
                                if dt < N_PE_DT:
                                    p = scanw.tile([128, TOK], dt_of("P"), tag="p", name="p", bufs=sbufs3)
                                    if not reverse:
                                        nc.vector.tensor_mul(p[:], h[:], C_rep[:])
                                    else:
                                        for b in range(BL):
                                            bs = slice(b * L, (b + 1) * L)
                                            nc.vector.tensor_mul(
                                                p[:, bs], h[:, bs][:, ::-1],
                                                C_rep[:, bs])
                                    for fh in range(2):
                                        fs = slice(fh * 512, (fh + 1) * 512)
                                        nc.tensor.matmul(y_ps[dt][:, fs],idn[:],p[:, fs],
                                                         start=(n == 0), stop=(n == N - 1))
                                elif n == 0:
                                    if not reverse:
                                        nc.vector.tensor_mul(y_acc[dt][:], h[:], C_rep[:])
                                    else:
                                        for b in range(BL):
                                            bs = slice(b * L, (b + 1) * L)
                                            nc.vector.tensor_mul(
                                                y_acc[dt][:, bs],
                                                h[:, bs][:, ::-1], C_rep[:, bs])
                                else:
                                    p = scanw.tile([128, TOK], dt_of("P"), tag="p", name="p", bufs=sbufs3)
                                    if not reverse:
                                        nc.vector.tensor_mul(p[:], h[:], C_rep[:])
                                    else:
                                        for b in range(BL):
                                            bs = slice(b * L, (b + 1) * L)
                                            nc.vector.tensor_mul(
                                                p[:, bs], h[:, bs][:, ::-1],
                                                C_rep[:, bs])
                                    if cfg["ADDS"] == "pool":
                                        nc.gpsimd.tensor_add(y_acc[dt][:], y_acc[dt][:], p[:])
                                    else:
                                        nc.vector.tensor_add(y_acc[dt][:], y_acc[dt][:], p[:])

                    # ---- gate + out_proj + residual ----
                    if cfg["PROBE"] == "stop_scan":
                        return [xs[0], xs[1]]
                    # y = y_acc + Dp*xs, then gate by silu(z) — both in place on xs
                    g = xs
                    for dt in range(DT_TILES):
                        ysrc = y_ps[dt] if dt < N_PE_DT else y_acc[dt]
                        nc.vector.scalar_tensor_tensor(
                            xs[dt][:], xs[dt][:], Dp_t[:, dt, 0:1], ysrc[:],
                            ALU.mult, ALU.add)
                        nc.vector.tensor_mul(xs[dt][:], xs[dt][:], sz[dt][:])
                    if pe_pool_ctx is not None:
                        pe_pool_ctx.__exit__(None, None, None)
                    xout = []
                    with tc.tile_pool(name="po", bufs=3, space="PSUM") as po:
                        for m in range(MT):
                            t = pers.tile([128, TOK], BF16, tag=f"x{s}out{m}", name=f"x{s}out{m}")
                            for fh in range(2):
                                fs = slice(fh * 512, (fh + 1) * 512)
                                ps = po.tile([128, 512], F32, tag="po", name="po")
                                for ks in range(DT_TILES):
                                    nc.tensor.matmul(
                                        ps[:],outw_t[:, ks, m * 128:(m + 1) * 128],g[ks][:, fs], start=(ks == 0),
                                        stop=(ks == DT_TILES - 1))
                                nc.vector.tensor_add(t[:, fs], ps[:], xT[m][:, fs])
                            xout.append(t)
                    return xout

                if cfg["PROBE"] == "base":
                    x1 = None
                else:
                    x1 = mamba_layer("f", reverse=False)
                x2 = x1 if (cfg["PROBE"] in ("layer1", "base") or cfg["PROBE"].startswith("stop_")) else mamba_layer("b", reverse=True)

                if cfg["PROBE"] == "base":
                    for m in range(MT):
                        nc.gpsimd.dma_start(outT_d[m * 128:(m + 1) * 128, :], xn[m][:])
                    return
                if cfg["PROBE"] == "nohead" or cfg["PROBE"].startswith("stop_"):
                    for m in range(MT):
                        nc.gpsimd.dma_start(outT_d[m * 128:(m + 1) * 128, :], x1[m][:])
                    return
                # ---- head: relu(cat(x1,x2) @ proj_w + proj_b), residual, layernorm ----
                cat = x1 + x2
                xn2 = []
                with tc.tile_pool(name="ph", bufs=3, space="PSUM") as ph:
                    for m in range(MT):
                        x2n = pers.tile([128, TOK], F32, tag=f"xn2_{m}", name=f"xn2_{m}")
                        for fh in range(2):
                            fs = slice(fh * 512, (fh + 1) * 512)
                            ps = ph.tile([128, 512], F32, tag="ph", name="ph")
                            for ks in range(4):
                                nc.tensor.matmul(
                                    ps[:],projw_t[:, ks, m * 128:(m + 1) * 128],cat[ks][:, fs], start=(ks == 0), stop=(ks == 3))
                            t = work.tile([128, 512], F32, tag="yh", name="yh")
                            nc.scalar.activation(t[:], ps[:], AF.Relu,
                                                 bias=projb_t[:, m, 0:1])
                            nc.vector.tensor_add(x2n[:, fs], t[:], xT[m][:, fs])
                        xn2.append(x2n)

                with tc.tile_pool(name="pln", bufs=1, space="PSUM") as pln:
                    ones_col = wpool.tile([128, 1], F32, tag="ones_col2", name="ones_col2")
                    nc.vector.memset(ones_col[:], 1.0)
                    ones1 = wpool.tile([1, 128], F32, tag="ones1b", name="ones1b")
                    nc.vector.memset(ones1[:], 1.0)
                    mu_ps = pln.tile([1, TOK], F32, tag="mu", name="mu")
                    ss_ps = pln.tile([1, TOK], F32, tag="ss2", name="ss2")
                    for fh in range(2):
                        fs = slice(fh * 512, (fh + 1) * 512)
                        for m in range(MT):
                            nc.tensor.matmul(mu_ps[:, fs],ones_col[:],xn2[m][:, fs],
                                             start=(m == 0), stop=(m == MT - 1))
                            sq = work.tile([128, 512], F32, tag="sqtmp", name="ln_sq")
                            nc.scalar.square(sq[:], xn2[m][:, fs])
                            nc.tensor.matmul(ss_ps[:, fs],ones_col[:],sq[:],
                                             start=(m == 0), stop=(m == MT - 1))
                    mu_row = wpool.tile([1, TOK], F32, tag="mu_row", name="mu_row")
                    nc.scalar.mul(mu_row[:], mu_ps[:], 1.0 / DM)
                    # var = ss/DM - mu^2 (built in rstd_row, then rstd in place)
                    rstd_row = wpool.tile([1, TOK], F32, tag="rstd_row", name="rstd_row")
                    nc.scalar.mul(rstd_row[:], ss_ps[:], 1.0 / DM)
                    mu2 = work.tile([1, TOK], F32, tag="rowtmp", name="mu2")
                    nc.vector.tensor_mul(mu2[:], mu_row[:], mu_row[:])
                    nc.vector.tensor_sub(rstd_row[:], rstd_row[:], mu2[:])
                    eps2 = wpool.tile([1, 1], F32, tag="eps2", name="eps2")
                    nc.vector.memset(eps2[:], 1e-5)
                    nc.scalar.activation(rstd_row[:], rstd_row[:], AF.Ln, bias=eps2[:, 0:1])
                    nc.scalar.activation(rstd_row[:], rstd_row[:], AF.Exp, scale=-0.5)
                    mu_rep = pln.tile([128, TOK], F32, tag="mu_rep", name="mu_rep")
                    rs_rep = pln.tile([128, TOK], F32, tag="rs_rep2", name="rs_rep2")
                    for fh in range(2):
                        fs = slice(fh * 512, (fh + 1) * 512)
                        nc.tensor.matmul(mu_rep[:, fs],ones1[:],mu_row[:, fs],
                                         start=True, stop=True)
                        nc.tensor.matmul(rs_rep[:, fs],ones1[:],rstd_row[:, fs],
                                         start=True, stop=True)
                    for m in range(MT):
                        nc.vector.tensor_sub(xn2[m][:], xn2[m][:], mu_rep[:])
                        nc.vector.tensor_mul(xn2[m][:], xn2[m][:], rs_rep[:])
                        nc.scalar.activation(xn2[m][:], xn2[m][:], AF.Identity,
                                             bias=lnb_t[:, m, 0:1],
                                             scale=lng_t[:, m, 0:1])
                        nc.sync.dma_start(outT_d[m * 128:(m + 1) * 128, :], xn2[m][:])

            if loop_k > 1:
                with tc.For_i(0, loop_k, 1):
                    body()
            else:
                body()

    nc.compile()
    _BUILD_CACHE[key] = nc
    return nc


# ======================================================================
# host entry
# ======================================================================

def _make_in_maps(inputs):
    x = np.asarray(inputs["x"], F32_np)
    fw = _prep_layer_weights(inputs["fm_in"], inputs["fm_convw"], inputs["fm_convb"],
                             inputs["fm_xproj"], inputs["fm_dtw"], inputs["fm_dtb"],
                             inputs["fm_Alog"], inputs["fm_D"], inputs["fm_out"],
                             inputs["fm_norm"])
    bw = _prep_layer_weights(inputs["bm_in"], inputs["bm_convw"], inputs["bm_convb"],
                             inputs["bm_xproj"], inputs["bm_dtw"], inputs["bm_dtb"],
                             inputs["bm_Alog"], inputs["bm_D"], inputs["bm_out"],
                             inputs["bm_norm"])
    sh = _prep_shared_weights(inputs["proj_w"], inputs["proj_b"],
                              inputs["ln_g"], inputs["ln_b"])
    base = {}
    for s, w in (("f", fw), ("b", bw)):
        for k, v in w.items():
            if k in ("convbn", "convb", "inw", "convd", "xpw", "dtw", "dtb", "A", "Dp",
                     "outw"):
                base[f"{s}_{k}"] = v
    base["projw"] = sh["projw"]
    base["projb"] = sh["projb"]
    base["lng"] = sh["lng"]
    base["lnb"] = sh["lnb"]

    in_maps = []
    for c in range(NCORES):
        xc = x[c * BL:(c + 1) * BL]                       # (BL, L, DM)
        xTc = np.ascontiguousarray(xc.reshape(TOK, DM).T)  # (DM, TOK)
        m = dict(base)
        m["xT"] = xTc
        in_maps.append(m)
    return in_maps


def _unshard(results):
    outs = []
    for c in range(NCORES):
        oT = results[c]["outT"]                            # (DM, TOK)
        outs.append(np.ascontiguousarray(oT.T.reshape(BL, L, DM)))
    return np.concatenate(outs, axis=0).astype(F32_np)


def kernel(**inputs):
    from concourse import bass_utils
    nc = _build(loop_k=1)
    in_maps = _make_in_maps(inputs)
    res = bass_utils.run_bass_kernel_spmd(nc, in_maps, core_ids=list(range(NCORES)))
    return _unshard(res.results)



# revision 40
# speedup vs baseline: 1.5503x; 1.5503x over previous
"""Bidirectional Mamba block on 8 Trainium2 NeuronCores (Bass/Tile).

Data-parallel over batch: B=16 -> 2 per core; weights replicated; host gathers.
Per-core layout is feature-major ([feature_partitions, tokens]) with tokens =
batch-major concatenation of the 2 local sequences (t = b*512 + l).

Engines:
  PE   - all projections (weights stationary as lhsT), depthwise causal conv as
         accumulating diag-matmuls over shifted views, partition-broadcast of
         per-token B/C rows via one-hot selector matmuls, y-accumulation over
         the 16 SSM states via identity-matmul PSUM accumulation (3/4 d-tiles).
  ACT  - sigmoid for both silu gates; softplus = ln(exp(.)+1);
         rsqrt = exp(-0.5*ln(.)); dA_n = exp(delta * A_n) as ONE [128,4096]
         exp per state (A is d-independent here -> immediate scalar scale,
         verified host-side with per-partition fallback); PSUM->SBUF copies.
  DVE  - selective scan via tensor_tensor_scan. All 4 d-tiles are fused along
         the free dim ([128, 4*1024] "QUAD" tiles) so each state costs ONE
         bx-mult, ONE 4096-wide scan and ONE h*C mult; state resets at the 7
         interior segment boundaries are implemented by zeroing those dA
         columns (one strided ACT op). The backward layer reads dA/bx through
         reversed APs. DVE op COUNT is the binding constraint (each op costs
         ~dur + 260ns); per-op drains are not exposed, so bigger ops win.
  Phasing: prep(f), prep(b), scan(f), scan(b) -- layer b's projections are
         independent of layer f, so issuing them before f's DVE-bound scan
         lets PE/ACT fill the scan's idle capacity.
"""

import numpy as np

# ---- problem constants (hardcoded per contract) ----
B, L, DM = 16, 512, 256
DI, N, R, KC = 512, 16, 16, 4
NCORES = 8
BL = B // NCORES          # local batch
TOK = BL * L              # 1024 tokens per core
DT_TILES = DI // 128      # 4
MT = DM // 128            # 2
F32_np = np.float32

# ---- dtype knobs for the scan path ----
import ml_dtypes
BF16_np = ml_dtypes.bfloat16

CFG = dict(
    DA="bf16",     # dA (scan decay operand)
    DELTA="bf16",  # delta resident
    W="bf16",      # w = delta*xs (scan drive factor)
    H="bf16",      # scan output h
    REP="bf16",    # B_rep / C_rep broadcast tiles
    P="bf16",      # products h*C
    YACC="bf16",   # y accumulator (only the non-PE d-tile)
    SZ="bf16",     # silu(z) gate
    XS="bf16",     # conv-silu output / gate buffer
    SILU="sigmoid",  # "sigmoid" table or "exp"+reciprocal
    PROBE="",      # timing probes: shrink a stage's work (breaks numerics)
    ADDS="pe",     # y_acc adds: "pe" (psum identity-matmul), "dve", "pool"
    MERGE="0",     # "1": single [128,TOK] scan w/ zeroed dA at batch boundary
    SCANENG="dve",  # "dve" | "mix<k>": dt >= k scans on gpsimd
    MULENG="dve",  # "dve" | "mix": bx mults on gpsimd
    SPLIT="0",     # "1": issue scan-loop muls as 2x [128,512] (drain amortization)
    CHAIN="0",     # "256": scans as chained [128,256] pieces (drain < 266ns)
    BREP="pe",     # B/C broadcast: "pe" one-hot matmul + ACT copy, "dma" partition_broadcast
    XMCOPY="dve",  # xmpad PSUM->SBUF copy engine: "dve" | "act"
    QUAD="1",      # "1": fuse all 4 d-tiles into [128,4,TOK] tiles; one scan/mul per n
    ACONST="",     # "f:<16 floats>;b:<16 floats>": A rows constant across d ->
                   # one flat dA exp per n with immediate scale (set by host)
    PHASED="1",    # "1": prep_f, prep_b, scan_f, scan_b (overlap b-prep w/ f-scan)
    WIDE="1",      # "1": FD-1024 PSUM tiles in in_proj/conv/dt (fewer, larger ops)
    GATECOPY="dve",  # "act": stage y_ps through ACT copy before the gate stt
)

_BUILD_CACHE = {}


# ======================================================================
# host-side weight preparation
# ======================================================================

def _prep_layer_weights(inw, convw, convb, xprojw, dtw, dtb, Alog, Dp, outw, normw):
    """Fold/reshape one mamba layer's weights into device layouts."""
    out = {}
    # in_proj with rmsnorm weight folded into rows: [128, 2, 1024]
    w = (np.asarray(normw)[:, None] * np.asarray(inw)).astype(F32_np)
    out["inw"] = np.ascontiguousarray(w.reshape(2, 128, 2 * DI).transpose(1, 0, 2)).astype(BF16_np)
    # conv diag matrices: [128, 16(dt*4+k), 128]
    cd = np.zeros((128, DT_TILES * KC, 128), F32_np)
    cw = np.asarray(convw).astype(F32_np)  # (KC, 1, DI)
    for dt in range(DT_TILES):
        for k in range(KC):
            idx = np.arange(128)
            cd[idx, dt * KC + k, idx] = cw[k, 0, dt * 128 + idx]
    out["convd"] = np.ascontiguousarray(cd).astype(BF16_np)
    out["convbn"] = np.ascontiguousarray(
        (-np.asarray(convb).astype(F32_np)).reshape(DT_TILES, 128, 1).transpose(1, 0, 2))
    out["convb"] = np.ascontiguousarray(
        np.asarray(convb).astype(F32_np).reshape(DT_TILES, 128, 1).transpose(1, 0, 2))
    # xproj padded so delta_raw/B/C land at partitions 0/32/64: [128, 4, 96]
    xp = np.zeros((DI, 96), F32_np)
    xpw = np.asarray(xprojw).astype(F32_np)
    xp[:, 0:R] = xpw[:, 0:R]
    xp[:, 32:32 + N] = xpw[:, R:R + N]
    xp[:, 64:64 + N] = xpw[:, R + N:R + 2 * N]
    out["xpw"] = np.ascontiguousarray(xp.reshape(DT_TILES, 128, 96).transpose(1, 0, 2)).astype(BF16_np)
    out["dtw"] = np.ascontiguousarray(np.asarray(dtw).astype(F32_np)).astype(BF16_np)          # (16, 512)
    out["dtb"] = np.ascontiguousarray(
        np.asarray(dtb).astype(F32_np).reshape(DT_TILES, 128, 1).transpose(1, 0, 2))
    A = (-np.exp(np.asarray(Alog).astype(np.float64))).astype(F32_np)          # (512, 16)
    out["A"] = np.ascontiguousarray(A.reshape(DT_TILES, 128, N).transpose(1, 0, 2))
    out["Dp"] = np.ascontiguousarray(
        np.asarray(Dp).astype(F32_np).reshape(DT_TILES, 128, 1).transpose(1, 0, 2))
    out["outw"] = np.ascontiguousarray(
        np.asarray(outw).astype(F32_np).reshape(DT_TILES, 128, DM).transpose(1, 0, 2)).astype(BF16_np)
    return out


def _prep_shared_weights(proj_w, proj_b, ln_g, ln_b):
    out = {}
    out["projw"] = np.ascontiguousarray(
        np.asarray(proj_w).astype(F32_np).reshape(4, 128, DM).transpose(1, 0, 2)).astype(BF16_np)
    out["projb"] = np.ascontiguousarray(
        np.asarray(proj_b).astype(F32_np).reshape(MT, 128, 1).transpose(1, 0, 2))
    out["lng"] = np.ascontiguousarray(
        np.asarray(ln_g).astype(F32_np).reshape(MT, 128, 1).transpose(1, 0, 2))
    out["lnb"] = np.ascontiguousarray(
        np.asarray(ln_b).astype(F32_np).reshape(MT, 128, 1).transpose(1, 0, 2))
    return out


# ======================================================================
# device program
# ======================================================================

def _build(loop_k=1, cfg=None, variant="full"):
    cfg = dict(CFG if cfg is None else cfg)
    key = (loop_k, variant, tuple(sorted(cfg.items())))
    if key in _BUILD_CACHE:
        return _BUILD_CACHE[key]

    import concourse.bacc as bacc
    import concourse.mybir as mybir
    import concourse.tile as tile

    F32 = mybir.dt.float32
    BF16 = mybir.dt.bfloat16
    AF = mybir.ActivationFunctionType
    ALU = mybir.AluOpType
    AX = mybir.AxisListType

    def dt_of(kname):
        return F32 if cfg[kname] == "f32" else BF16

    nc = bacc.Bacc("TRN2", target_bir_lowering=False, debug=False)

    def din(name, shape, dt=None):
        return nc.dram_tensor(name, list(shape), dt or F32, kind="ExternalInput").ap()

    # --- DRAM I/O ---
    xT_d = din("xT", (DM, TOK), BF16)
    lw_d = {}
    for s in ("f", "b"):
        lw_d[s] = {
            "inw": din(f"{s}_inw", (128, 2, 2 * DI), BF16),
            "convd": din(f"{s}_convd", (128, DT_TILES * KC, 128), BF16),
            "convbn": din(f"{s}_convbn", (128, DT_TILES, 1)),
            "convb": din(f"{s}_convb", (128, DT_TILES, 1)),
            "xpw": din(f"{s}_xpw", (128, DT_TILES, 96), BF16),
            "dtw": din(f"{s}_dtw", (16, DI), BF16),
            "dtb": din(f"{s}_dtb", (128, DT_TILES, 1)),
            "A": din(f"{s}_A", (128, DT_TILES, N)),
            "Dp": din(f"{s}_Dp", (128, DT_TILES, 1)),
            "outw": din(f"{s}_outw", (128, DT_TILES, DM), BF16),
        }
    projw_d = din("projw", (128, 4, DM), BF16)
    projb_d = din("projb", (128, MT, 1))
    lng_d = din("lng", (128, MT, 1))
    lnb_d = din("lnb", (128, MT, 1))
    outT_d = nc.dram_tensor("outT", [DM, TOK], F32, kind="ExternalOutput").ap()

    PAD = KC - 1  # 3
    CONVW = 2 * PAD + L  # padded per-batch row length 518

    with tile.TileContext(nc) as tc:
        from contextlib import ExitStack
        with ExitStack() as ctx:
            wpool = ctx.enter_context(tc.tile_pool(name="wpool", bufs=1))
            pers = ctx.enter_context(tc.tile_pool(name="pers", bufs=1))
            work = ctx.enter_context(tc.tile_pool(name="work", bufs=2))
            rep = ctx.enter_context(tc.tile_pool(name="rep", bufs=2 if cfg["ADDS"] == "pe" else 1))
            scanw = ctx.enter_context(tc.tile_pool(name="scanw", bufs=2))
            sbufs3 = 3 if cfg.get("LOOKAHEAD") == "3" else None

            def body():
                # ---- load shared weights ----
                projw_t = wpool.tile([128, 4, DM], BF16, tag="projw", name="projw")
                nc.sync.dma_start(projw_t[:], projw_d[:])
                projb_t = wpool.tile([128, MT, 1], F32, tag="projb", name="projb")
                nc.sync.dma_start(projb_t[:], projb_d[:])
                lng_t = wpool.tile([128, MT, 1], F32, tag="lng", name="lng")
                nc.sync.dma_start(lng_t[:], lng_d[:])
                lnb_t = wpool.tile([128, MT, 1], F32, tag="lnb", name="lnb")
                nc.sync.dma_start(lnb_t[:], lnb_d[:])

                xT = []
                for m in range(MT):
                    t = pers.tile([128, TOK], BF16, tag=f"xT{m}", name=f"xT{m}")
                    nc.sync.dma_start(t[:], xT_d[m * 128:(m + 1) * 128, :])
                    xT.append(t)

                # ---- shared RMSNorm: xn = x * rsqrt(mean(x^2) + eps) ----
                xn = []
                with tc.tile_pool(name="prms", bufs=1, space="PSUM") as prms:
                    ones_col = wpool.tile([128, 1], F32, tag="ones_col", name="ones_col")
                    nc.vector.memset(ones_col[:], 1.0)
                    ss_ps = prms.tile([1, TOK], F32, tag="ss", name="ss")
                    for fh in range(2):
                        fs = slice(fh * 512, (fh + 1) * 512)
                        for m in range(MT):
                            sq = work.tile([128, 512], F32, tag="sqtmp", name="rms_sq", bufs=1)
                            nc.scalar.square(sq[:], xT[m][:, fs])
                            nc.tensor.matmul(ss_ps[:, fs],ones_col[:],sq[:],
                                             start=(m == 0), stop=(m == MT - 1))
                    # rs = exp(-0.5 * ln(ss/DM + eps))
                    eps1 = wpool.tile([1, 1], F32, tag="eps1", name="eps1")
                    nc.vector.memset(eps1[:], 1e-5)
                    rs_row = work.tile([1, TOK], F32, tag="rowtmp", name="rs_row", bufs=1)
                    nc.scalar.activation(rs_row[:], ss_ps[:], AF.Ln,
                                         scale=1.0 / DM, bias=eps1[:, 0:1])
                    nc.scalar.activation(rs_row[:], rs_row[:], AF.Exp, scale=-0.5)
                    ones1 = wpool.tile([1, 128], F32, tag="ones1", name="ones1")
                    nc.vector.memset(ones1[:], 1.0)
                    rs_ps = prms.tile([128, TOK], F32, tag="rs_rep", name="rs_rep")
                    for fh in range(2):
                        fs = slice(fh * 512, (fh + 1) * 512)
                        nc.tensor.matmul(rs_ps[:, fs],ones1[:],rs_row[:, fs],
                                         start=True, stop=True)
                    for m in range(MT):
                        t = pers.tile([128, TOK], BF16, tag=f"xn{m}", name=f"xn{m}")
                        nc.vector.tensor_mul(t[:], xT[m][:], rs_ps[:])
                        xn.append(t)

                quad = cfg["QUAD"] == "1"
                if cfg["MERGE"] == "1" or quad:
                    negc = wpool.tile([128, 1], F32, tag="negc", name="negc")
                    nc.vector.memset(negc[:], -90.0)
                else:
                    negc = None
                if cfg["ADDS"] in ("pe", "dma"):
                    idn = wpool.tile([128, 128], BF16, tag="idn", name="idn")
                    from concourse.masks import make_identity
                    make_identity(nc, idn[:])

                # ---- one mamba layer: prep (projections) then scan ----
                def mamba_prep(s, reverse):
                    W = lw_d[s]
                    a_const = None
                    if cfg["ACONST"]:
                        for part in cfg["ACONST"].split(";"):
                            tag_, vals = part.split(":")
                            if tag_ == s:
                                a_const = [float(x) for x in vals.split()]
                    inw_t = wpool.tile([128, 2, 2 * DI], BF16, tag="inw", name="inw")
                    nc.sync.dma_start(inw_t[:], W["inw"][:])
                    convd_t = wpool.tile([128, DT_TILES * KC, 128], BF16, tag="convd", name="convd")
                    nc.sync.dma_start(convd_t[:], W["convd"][:])
                    convbn_t = wpool.tile([128, DT_TILES, 1], F32, tag="convbn", name="convbn")
                    nc.sync.dma_start(convbn_t[:], W["convbn"][:])
                    convb_t = wpool.tile([128, DT_TILES, 1], F32, tag="convb", name="convb")
                    nc.sync.dma_start(convb_t[:], W["convb"][:])
                    xpw_t = wpool.tile([128, DT_TILES, 96], BF16, tag="xpw", name="xpw")
                    nc.sync.dma_start(xpw_t[:], W["xpw"][:])
                    dtw_t = wpool.tile([16, DI], BF16, tag="dtw", name="dtw")
                    nc.sync.dma_start(dtw_t[:], W["dtw"][:])
                    dtb_t = wpool.tile([128, DT_TILES, 1], F32, tag="dtb", name="dtb")
                    nc.sync.dma_start(dtb_t[:], W["dtb"][:])
                    A_t = wpool.tile([128, DT_TILES, N], F32, tag=f"{s}_A", name="A")
                    nc.sync.dma_start(A_t[:], W["A"][:])
                    Dp_t = wpool.tile([128, DT_TILES, 1], F32, tag=f"{s}_Dp", name="Dp")
                    nc.sync.dma_start(Dp_t[:], W["Dp"][:])
                    outw_t = wpool.tile([128, DT_TILES, DM], BF16, tag=f"{s}_outw", name="outw")
                    nc.sync.dma_start(outw_t[:], W["outw"][:])

                    xmpad = []
                    sz = []
                    xs = []
                    sz_q = xs_q = delta_q = w_q = None
                    if quad:
                        sz_q = pers.tile([128, DT_TILES, TOK], dt_of("SZ"), tag=f"{s}_sz_q", name="sz_q")
                        xs_q = pers.tile([128, DT_TILES, TOK], dt_of("XS"), tag=f"{s}_xs_q", name="xs_q")
                        sz = [sz_q[:, dt, :] for dt in range(DT_TILES)]
                        xs = [xs_q[:, dt, :] for dt in range(DT_TILES)]
                    for dt in range(DT_TILES):
                        t = pers.tile([128, BL, CONVW], BF16, tag=f"xmpad{dt}", name=f"xmpad{dt}")
                        nc.vector.memset(t[:, :, 0:PAD], 0.0)
                        nc.vector.memset(t[:, :, PAD + L:CONVW], 0.0)
                        xmpad.append(t)
                        if not quad:
                            sz.append(pers.tile([128, TOK], dt_of("SZ"), tag=f"{s}_sz{dt}", name=f"sz{dt}"))
                            xs.append(pers.tile([128, TOK], dt_of("XS"), tag=f"{s}_xs{dt}", name=f"xs{dt}"))

                    # ---- in_proj ----
                    wide = cfg["WIDE"] == "1"
                    with tc.tile_pool(name="pp", bufs=4, space="PSUM") as pp:
                        for m in range(8):
                            if wide:
                                ps = pp.tile([128, TOK], F32, tag="pp", name="pp")
                                for fh in range(2):
                                    fs = slice(fh * 512, (fh + 1) * 512)
                                    for ks in range(2):
                                        nc.tensor.matmul(
                                            ps[:, fs],inw_t[:, ks, m * 128:(m + 1) * 128],xn[ks][:, fs], start=(ks == 0), stop=(ks == 1))
                                if m < 4:
                                    src3 = ps[:].rearrange("p (a b) -> p a b", a=BL)
                                    if cfg["XMCOPY"] == "act":
                                        nc.scalar.copy(xmpad[m][:, :, PAD:PAD + L], src3)
                                    else:
                                        nc.vector.tensor_copy(xmpad[m][:, :, PAD:PAD + L], src3)
                                else:
                                    zdt = m - 4
                                    e = work.tile([128, TOK], BF16, tag="zetag", name="ze")
                                    nc.scalar.activation(e[:], ps[:], AF.Sigmoid)
                                    nc.vector.tensor_mul(sz[zdt][:], e[:], ps[:])
                                continue
                            for fh in range(2):
                                fs = slice(fh * 512, (fh + 1) * 512)
                                ps = pp.tile([128, 512], F32, tag="pp", name="pp")
                                for ks in range(2):
                                    nc.tensor.matmul(
                                        ps[:],inw_t[:, ks, m * 128:(m + 1) * 128],xn[ks][:, fs], start=(ks == 0), stop=(ks == 1))
                                if m < 4:
                                    # xm -> padded conv buffer (fh == local batch idx)
                                    if cfg["XMCOPY"] == "act":
                                        nc.scalar.copy(xmpad[m][:, fh, PAD:PAD + L], ps[:])
                                    else:
                                        nc.vector.tensor_copy(xmpad[m][:, fh, PAD:PAD + L], ps[:])
                                else:
                                    zdt = m - 4
                                    e = work.tile([128, 512], F32, tag="zetag", name="ze")
                                    if cfg["SILU"] == "sigmoid":
                                        nc.scalar.activation(e[:], ps[:], AF.Sigmoid)
                                    else:
                                        nc.scalar.activation(e[:], ps[:], AF.Exp, scale=-1.0)
                                        nc.vector.tensor_scalar(e[:], e[:], 1.0, None, ALU.add)
                                        nc.vector.reciprocal(e[:], e[:])
                                    nc.vector.tensor_mul(sz[zdt][:, fs], e[:], ps[:])

                        # ---- depthwise causal conv + silu ----
                        for dt in range(DT_TILES):
                            if wide:
                                ps = pp.tile([128, TOK], F32, tag="pp", name="pp")
                                for b in range(BL):
                                    for k in range(KC):
                                        off = k if not reverse else (2 * PAD - k)
                                        nc.tensor.matmul(
                                            ps[:, b * L:(b + 1) * L],convd_t[:, dt * KC + k, :],
                                            xmpad[dt][:, b, off:off + L],
                                            start=(k == 0), stop=(k == KC - 1))
                                e = work.tile([128, TOK], BF16, tag="cetag", name="ce")
                                nc.scalar.activation(e[:], ps[:], AF.Sigmoid,
                                                     bias=convb_t[:, dt, 0:1])
                                nc.vector.scalar_tensor_tensor(
                                    xs[dt][:], ps[:], convb_t[:, dt, 0:1], e[:],
                                    ALU.add, ALU.mult)
                                continue
                            for b in range(BL):
                                ps = pp.tile([128, 512], F32, tag="pp", name="pp")
                                for k in range(KC):
                                    off = k if not reverse else (2 * PAD - k)
                                    nc.tensor.matmul(
                                        ps[:],convd_t[:, dt * KC + k, :],xmpad[dt][:, b, off:off + L],
                                        start=(k == 0), stop=(k == KC - 1))
                                bs = slice(b * L, (b + 1) * L)
                                e = work.tile([128, 512], F32, tag="cetag", name="ce")
                                if cfg["SILU"] == "sigmoid":
                                    nc.scalar.activation(e[:], ps[:], AF.Sigmoid,
                                                         bias=convb_t[:, dt, 0:1])
                                else:
                                    nc.scalar.activation(e[:], ps[:], AF.Exp, scale=-1.0,
                                                         bias=convbn_t[:, dt, 0:1])
                                    nc.vector.tensor_scalar(e[:], e[:], 1.0, None, ALU.add)
                                    nc.vector.reciprocal(e[:], e[:])
                                # xs = (conv + bias) * sigmoid  (silu)
                                nc.vector.scalar_tensor_tensor(
                                    xs[dt][:, bs], ps[:], convb_t[:, dt, 0:1], e[:],
                                    ALU.add, ALU.mult)

                    if cfg["PROBE"] == "stop_conv":
                        return dict(stop=[xs[0], xs[1]])
                    # ---- xproj -> delta_raw / Brows / Crows ----
                    dbc = pers.tile([16, 2, TOK], BF16, tag=f"{s}_dbc", name="dbc")
                    draw_t = work.tile([16, TOK], BF16, tag="draw", name="draw_t", bufs=1)
                    draw = draw_t[:, :]
                    Brows = dbc[:, 0, :]
                    Crows = dbc[:, 1, :]
                    with tc.tile_pool(name="pxp", bufs=1, space="PSUM") as pxp:
                        psx = pxp.tile([96, TOK], F32, tag="pxp", name="pxp")
                        for fh in range(2):
                            fs = slice(fh * 512, (fh + 1) * 512)
                            for ks in range(DT_TILES):
                                nc.tensor.matmul(psx[:, fs],xpw_t[:, ks, :],xs[ks][:, fs],
                                                 start=(ks == 0), stop=(ks == DT_TILES - 1))
                        nc.scalar.copy(draw, psx[0:16, :])
                        nc.scalar.copy(Brows, psx[32:48, :])
                        nc.scalar.copy(Crows, psx[64:80, :])

                    # ---- dt_proj + softplus -> delta; w = delta * xs ----
                    delta = []
                    w_t = []
                    with tc.tile_pool(name="pdt", bufs=3, space="PSUM") as pdt:
                        if quad:
                            delta_q = pers.tile([128, DT_TILES, TOK], dt_of("DELTA"), tag=f"{s}_delta_q", name="delta_q")
                            w_q = pers.tile([128, DT_TILES, TOK], dt_of("W"), tag=f"{s}_w_q", name="w_q")
                            delta = [delta_q[:, dt, :] for dt in range(DT_TILES)]
                            w_t = [w_q[:, dt, :] for dt in range(DT_TILES)]
                        for dt in range(DT_TILES):
                            dl = delta[dt] if quad else pers.tile(
                                [128, TOK], dt_of("DELTA"), tag=f"{s}_delta{dt}", name=f"delta{dt}")
                            for fh in range(2):
                                fs = slice(fh * 512, (fh + 1) * 512)
                                ps = pdt.tile([128, 512], F32, tag="pdt", name="pdt")
                                nc.tensor.matmul(ps[:],dtw_t[:, dt * 128:(dt + 1) * 128],draw[:, fs], start=True, stop=True)
                                e = work.tile([128, 512], F32, tag="detag", name="de")
                                nc.scalar.activation(e[:], ps[:], AF.Exp,
                                                     bias=dtb_t[:, dt, 0:1])
                                nc.scalar.activation(dl[:, fs], e[:], AF.Ln, bias=1.0)
                            if not quad:
                                delta.append(dl)
                                wt = pers.tile([128, TOK], dt_of("W"), tag=f"{s}_w{dt}", name=f"w{dt}")
                                nc.vector.tensor_mul(wt[:], dl[:], xs[dt][:])
                                w_t.append(wt)
                        if quad:
                            nc.vector.tensor_mul(w_q[:, :, :], delta_q[:, :, :], xs_q[:, :, :])

                    if cfg["PROBE"] == "stop_dt":
                        return dict(stop=[xs[0], xs[1]])
                    return dict(s=s, reverse=reverse, a_const=a_const, quad=quad,
                                delta=delta, delta_q=delta_q, w_t=w_t, w_q=w_q,
                                xs=xs, xs_q=xs_q, sz=sz, sz_q=sz_q,
                                Brows=Brows, Crows=Crows, A_t=A_t, Dp_t=Dp_t,
                                outw_t=outw_t)

                def mamba_scan(st):
                    (s, reverse, a_const, delta, delta_q, w_t, w_q, xs, xs_q, sz, sz_q,
                     Brows, Crows, A_t, Dp_t, outw_t) = (
                        st["s"], st["reverse"], st["a_const"], st["delta"], st["delta_q"],
                        st["w_t"], st["w_q"], st["xs"], st["xs_q"], st["sz"], st["sz_q"],
                        st["Brows"], st["Crows"], st["A_t"], st["Dp_t"], st["outw_t"])
                    # ---- selective scan ----
                    use_pe = cfg["ADDS"] in ("pe", "dma")
                    N_PE_DT = (4 if cfg["BREP"] == "dma" else 3) if use_pe else 0
                    y_acc = [None if dt < N_PE_DT else
                             pers.tile([128, TOK], dt_of("YACC"), tag=f"yacc{dt}", name=f"yacc{dt}")
                             for dt in range(DT_TILES)]
                    pe_pool_ctx = tc.tile_pool(name="pyac", bufs=1, space="PSUM") if use_pe else None
                    if pe_pool_ctx is not None:
                        pyac = pe_pool_ctx.__enter__()
                        y_ps = [pyac.tile([128, TOK], F32, tag=f"yps{dt}", name=f"yps{dt}")
                                for dt in range(N_PE_DT)]
                    with tc.tile_pool(name="prep", bufs=1, space="PSUM") as prep:
                        for n in range(N):
                            B_rep = rep.tile([128, TOK], dt_of("REP"), tag="B_rep", name="B_rep")
                            C_rep = rep.tile([128, TOK], dt_of("REP"), tag="C_rep", name="C_rep")
                            if cfg["BREP"] == "dma":
                                nc.gpsimd.partition_broadcast(B_rep[:], Brows[n:n + 1, :])
                                nc.gpsimd.partition_broadcast(C_rep[:], Crows[n:n + 1, :])
                            else:
                                seln = work.tile([16, 128], BF16, tag="seln", name="seln")
                                nc.gpsimd.memset(seln[:], 0.0)
                                nc.gpsimd.affine_select(
                                    out=seln[:], in_=seln[:], compare_op=ALU.not_equal,
                                    fill=1.0, base=-n, pattern=[[0, 128]],
                                    channel_multiplier=1)
                                for fh in range(2):
                                    fs = slice(fh * 512, (fh + 1) * 512)
                                    psB = prep.tile([128, 512], F32, tag="psB", name="psB", bufs=1)
                                    nc.tensor.matmul(psB[:], seln[:], Brows[:, fs], start=True, stop=True)
                                    nc.scalar.copy(B_rep[:, fs], psB[:])
                                    psC = prep.tile([128, 512], F32, tag="psC", name="psC", bufs=1)
                                    nc.tensor.matmul(psC[:], seln[:], Crows[:, fs], start=True, stop=True)
                                    nc.scalar.copy(C_rep[:, fs], psC[:])

                            if quad:
                                dAq = scanw.tile([128, DT_TILES, TOK], dt_of("DA"), tag="dA", name="dA")
                                dAf = dAq[:, :, :].rearrange("p a b -> p (a b)")
                                deltaf = delta_q[:, :, :].rearrange("p a b -> p (a b)")
                                if a_const is not None:
                                    nc.scalar.activation(dAf, deltaf, AF.Exp,
                                                         scale=float(a_const[n]))
                                else:
                                    for dt in range(DT_TILES):
                                        nc.scalar.activation(dAq[:, dt, :], delta[dt], AF.Exp,
                                                             scale=A_t[:, dt, n:n + 1])
                                # zero decay at the 7 interior segment starts (scan order)
                                bsl = slice(512, 3585, 512) if not reverse else slice(511, 3584, 512)
                                nc.scalar.activation(dAf[:, bsl], deltaf[:, bsl], AF.Exp,
                                                     scale=0.0, bias=negc[:, 0:1])
                                B3 = B_rep[:].unsqueeze(1).broadcast_to([128, DT_TILES, TOK])
                                C3 = C_rep[:].unsqueeze(1).broadcast_to([128, DT_TILES, TOK])
                                bxq = scanw.tile([128, DT_TILES, TOK], dt_of("W"), tag="bx", name="bx", bufs=1)
                                nc.vector.tensor_mul(bxq[:, :, :], w_q[:, :, :], B3)
                                hq = scanw.tile([128, DT_TILES, TOK], dt_of("H"), tag="h", name="h", bufs=1)
                                hf = hq[:, :, :].rearrange("p a b -> p (a b)")
                                bxf = bxq[:, :, :].rearrange("p a b -> p (a b)")
                                if not reverse:
                                    nc.vector.tensor_tensor_scan(hf, dAf, bxf, 0.0,
                                                                 ALU.mult, ALU.add)
                                else:
                                    nc.vector.tensor_tensor_scan(hf, dAf[:, ::-1], bxf[:, ::-1],
                                                                 0.0, ALU.mult, ALU.add)
                                pq = scanw.tile([128, DT_TILES, TOK], dt_of("P"), tag="p", name="p", bufs=1)
                                if not reverse:
                                    nc.vector.tensor_mul(pq[:, :, :], hq[:, :, :], C3)
                                else:
                                    nc.vector.tensor_mul(pq[:, :, :], hq[:, ::-1, ::-1], C3)
                                for dt in range(N_PE_DT):
                                    for fh in range(2):
                                        fs = slice(fh * 512, (fh + 1) * 512)
                                        nc.tensor.matmul(y_ps[dt][:, fs], idn[:], pq[:, dt, fs],
                                                         start=(n == 0), stop=(n == N - 1))
                                for dt in range(N_PE_DT, DT_TILES):
                                    if cfg["ADDS"] == "dma":
                                        if n == 0:
                                            nc.gpsimd.dma_start(y_acc[dt][:], pq[:, dt, :])
                                        else:
                                            nc.gpsimd.dma_start(y_acc[dt][:], pq[:, dt, :],
                                                              accum_op=ALU.add)
                                    elif n == 0:
                                        nc.vector.tensor_copy(y_acc[dt][:], pq[:, dt, :])
                                    else:
                                        nc.vector.tensor_add(y_acc[dt][:], y_acc[dt][:], pq[:, dt, :])
                                continue

                            merged = cfg["MERGE"] == "1"
                            # engine routers for the scan inner loop
                            def scan_eng(dt):
                                se = cfg["SCANENG"]
                                if se.startswith("mix") and dt >= int(se[3:]):
                                    return nc.gpsimd
                                return nc.vector

                            def mul_eng(dt):
                                me = cfg["MULENG"]
                                if me == "pool":
                                    return nc.gpsimd
                                if me.startswith("mix") and dt >= int(me[3:]):
                                    return nc.gpsimd
                                return nc.vector

                            for dt in range(DT_TILES):
                                dA = scanw.tile([128, TOK], dt_of("DA"), tag="dA", name="dA")
                                _dsl = slice(0, 64) if cfg["PROBE"] in ("dA", "acts") else slice(0, TOK)
                                nc.scalar.activation(dA[:, _dsl], delta[dt][:, _dsl], AF.Exp,
                                                     scale=A_t[:, dt, n:n + 1])
                                if merged:
                                    # zero decay at the scan-order batch boundary so one
                                    # [128,TOK] scan resets state between local batches
                                    bc = L if not reverse else L - 1
                                    nc.scalar.activation(dA[:, bc:bc + 1], delta[dt][:, bc:bc + 1],
                                                         AF.Exp, scale=0.0, bias=negc[:, 0:1])
                                bx = scanw.tile([128, TOK], dt_of("W"), tag="bx", name="bx", bufs=sbufs3)
                                if cfg["PROBE"] == "tt":
                                    mul_eng(dt).tensor_mul(bx[:, 0:64], w_t[dt][:, 0:64], B_rep[:, 0:64])
                                elif cfg["SPLIT"] == "1":
                                    for fh in range(2):
                                        fs = slice(fh * 512, (fh + 1) * 512)
                                        mul_eng(dt).tensor_mul(bx[:, fs], w_t[dt][:, fs], B_rep[:, fs])
                                else:
                                    mul_eng(dt).tensor_mul(bx[:], w_t[dt][:], B_rep[:])
                                h = scanw.tile([128, TOK], dt_of("H"), tag="h", name="h", bufs=sbufs3)
                                if variant == "noscan":
                                    nc.vector.tensor_mul(h[:], dA[:], bx[:])
                                elif merged:
                                    _sl = 64 if cfg["PROBE"] == "scan64" else TOK
                                    if not reverse:
                                        scan_eng(dt).tensor_tensor_scan(
                                            h[:, 0:_sl], dA[:, 0:_sl], bx[:, 0:_sl], 0.0,
                                            ALU.mult, ALU.add)
                                    else:
                                        scan_eng(dt).tensor_tensor_scan(
                                            h[:, 0:_sl], dA[:, ::-1][:, 0:_sl],
                                            bx[:, ::-1][:, 0:_sl], 0.0,
                                            ALU.mult, ALU.add)
                                else:
                                    for b in range(BL):
                                        bs = slice(b * L, (b + 1) * L)
                                        if not reverse:
                                            scan_eng(dt).tensor_tensor_scan(
                                                h[:, bs], dA[:, bs], bx[:, bs], 0.0,
                                                ALU.mult, ALU.add)
                                        else:
                                            scan_eng(dt).tensor_tensor_scan(
                                                h[:, bs], dA[:, bs][:, ::-1],
                                                bx[:, bs][:, ::-1], 0.0,
                                                ALU.mult, ALU.add)

                                def cmul(dst, eng):
                                    # dst = h * C_rep with h read back in layer order
                                    _psl = slice(0, 64) if cfg["PROBE"] == "ps64" else slice(0, TOK)
                                    if cfg["PROBE"] == "ps64":
                                        eng.tensor_mul(dst[:, _psl], h[:, _psl], C_rep[:, _psl])
                                    elif not reverse:
                                        if cfg["SPLIT"] == "1":
                                            for b in range(BL):
                                                bs = slice(b * L, (b + 1) * L)
                                                eng.tensor_mul(dst[:, bs], h[:, bs], C_rep[:, bs])
                                        else:
                                            eng.tensor_mul(dst[:], h[:], C_rep[:])
                                    elif merged:
                                        eng.tensor_mul(dst[:], h[:, ::-1], C_rep[:])
                                    else:
                                        for b in range(BL):
                                            bs = slice(b * L, (b + 1) * L)
                                            eng.tensor_mul(dst[:, bs], h[:, bs][:, ::-1],
                                                           C_rep[:, bs])

                                if dt < N_PE_DT:
                                    p = scanw.tile([128, TOK], dt_of("P"), tag="p", name="p", bufs=sbufs3)
                                    cmul(p, mul_eng(dt))
                                    for fh in range(2):
                                        fs = slice(fh * 512, (fh + 1) * 512)
                                        nc.tensor.matmul(y_ps[dt][:, fs],idn[:],p[:, fs],
                                                         start=(n == 0), stop=(n == N - 1))
                                elif n == 0:
                                    cmul(y_acc[dt], mul_eng(dt))
                                else:
                                    p = scanw.tile([128, TOK], dt_of("P"), tag="p", name="p", bufs=sbufs3)
                                    cmul(p, mul_eng(dt))
                                    if cfg["ADDS"] == "pool":
                                        nc.gpsimd.tensor_add(y_acc[dt][:], y_acc[dt][:], p[:])
                                    elif cfg["SPLIT"] == "1":
                                        for b in range(BL):
                                            bs = slice(b * L, (b + 1) * L)
                                            nc.vector.tensor_add(y_acc[dt][:, bs], y_acc[dt][:, bs], p[:, bs])
                                    else:
                                        nc.vector.tensor_add(y_acc[dt][:], y_acc[dt][:], p[:])

                    # ---- gate + out_proj + residual ----
                    if cfg["PROBE"] == "stop_scan":
                        return [xs[0], xs[1]]
                    # y = y_acc + Dp*xs, then gate by silu(z) — both in place on xs
                    g = xs
                    for dt in range(DT_TILES):
                        ysrc = y_ps[dt] if dt < N_PE_DT else y_acc[dt]
                        if cfg["GATECOPY"] == "act" and dt < N_PE_DT:
                            yc = work.tile([128, TOK], BF16, tag="ycop", name="ycop", bufs=1)
                            nc.scalar.copy(yc[:], ysrc[:])
                            ysrc = yc
                        nc.vector.scalar_tensor_tensor(
                            xs[dt][:], xs[dt][:], Dp_t[:, dt, 0:1], ysrc[:],
                            ALU.mult, ALU.add)
                        if not quad:
                            nc.vector.tensor_mul(xs[dt][:], xs[dt][:], sz[dt][:])
                    if quad:
                        nc.vector.tensor_mul(xs_q[:, :, :], xs_q[:, :, :], sz_q[:, :, :])
                    if pe_pool_ctx is not None:
                        pe_pool_ctx.__exit__(None, None, None)
                    xout = []
                    with tc.tile_pool(name="po", bufs=3, space="PSUM") as po:
                        for m in range(MT):
                            t = pers.tile([128, TOK], BF16, tag=f"x{s}out{m}", name=f"x{s}out{m}")
                            for fh in range(2):
                                fs = slice(fh * 512, (fh + 1) * 512)
                                ps = po.tile([128, 512], F32, tag="po", name="po")
                                for ks in range(DT_TILES):
                                    nc.tensor.matmul(
                                        ps[:],outw_t[:, ks, m * 128:(m + 1) * 128],g[ks][:, fs], start=(ks == 0),
                                        stop=(ks == DT_TILES - 1))
                                nc.vector.tensor_add(t[:, fs], ps[:], xT[m][:, fs])
                            xout.append(t)
                    return xout

                def mamba_layer(s, reverse):
                    st = mamba_prep(s, reverse)
                    if "stop" in st:
                        return st["stop"]
                    return mamba_scan(st)

                if cfg["PROBE"] == "base":
                    x1 = None
                    x2 = None
                elif cfg["PHASED"] == "1" and not (
                        cfg["PROBE"] in ("layer1",) or cfg["PROBE"].startswith("stop_")):
                    stf = mamba_prep("f", False)
                    stb = mamba_prep("b", True)
                    x1 = mamba_scan(stf)
                    x2 = mamba_scan(stb)
                else:
                    x1 = mamba_layer("f", reverse=False)
                    x2 = x1 if (cfg["PROBE"] in ("layer1", "base") or cfg["PROBE"].startswith("stop_")) else mamba_layer("b", reverse=True)

                if cfg["PROBE"] == "base":
                    for m in range(MT):
                        nc.gpsimd.dma_start(outT_d[m * 128:(m + 1) * 128, :], xn[m][:])
                    return
                if cfg["PROBE"] == "nohead" or cfg["PROBE"].startswith("stop_"):
                    for m in range(MT):
                        nc.gpsimd.dma_start(outT_d[m * 128:(m + 1) * 128, :], x1[m][:])
                    return
                # ---- head: relu(cat(x1,x2) @ proj_w + proj_b), residual, layernorm ----
                cat = x1 + x2
                xn2 = []
                with tc.tile_pool(name="ph", bufs=3, space="PSUM") as ph:
                    for m in range(MT):
                        x2n = pers.tile([128, TOK], F32, tag=f"xn2_{m}", name=f"xn2_{m}")
                        for fh in range(2):
                            fs = slice(fh * 512, (fh + 1) * 512)
                            ps = ph.tile([128, 512], F32, tag="ph", name="ph")
                            for ks in range(4):
                                nc.tensor.matmul(
                                    ps[:],projw_t[:, ks, m * 128:(m + 1) * 128],cat[ks][:, fs], start=(ks == 0), stop=(ks == 3))
                            t = work.tile([128, 512], F32, tag="yh", name="yh")
                            nc.scalar.activation(t[:], ps[:], AF.Relu,
                                                 bias=projb_t[:, m, 0:1])
                            nc.vector.tensor_add(x2n[:, fs], t[:], xT[m][:, fs])
                        xn2.append(x2n)

                with tc.tile_pool(name="pln", bufs=1, space="PSUM") as pln:
                    ones_col = wpool.tile([128, 1], F32, tag="ones_col2", name="ones_col2")
                    nc.vector.memset(ones_col[:], 1.0)
                    ones1 = wpool.tile([1, 128], F32, tag="ones1b", name="ones1b")
                    nc.vector.memset(ones1[:], 1.0)
                    mu_ps = pln.tile([1, TOK], F32, tag="mu", name="mu")
                    ss_ps = pln.tile([1, TOK], F32, tag="ss2", name="ss2")
                    for fh in range(2):
                        fs = slice(fh * 512, (fh + 1) * 512)
                        for m in range(MT):
                            nc.tensor.matmul(mu_ps[:, fs],ones_col[:],xn2[m][:, fs],
                                             start=(m == 0), stop=(m == MT - 1))
                            sq = work.tile([128, 512], F32, tag="sqtmp", name="ln_sq", bufs=1)
                            nc.scalar.square(sq[:], xn2[m][:, fs])
                            nc.tensor.matmul(ss_ps[:, fs],ones_col[:],sq[:],
                                             start=(m == 0), stop=(m == MT - 1))
                    mu_row = wpool.tile([1, TOK], F32, tag="mu_row", name="mu_row")
                    nc.scalar.mul(mu_row[:], mu_ps[:], 1.0 / DM)
                    # var = ss/DM - mu^2 (built in rstd_row, then rstd in place)
                    rstd_row = wpool.tile([1, TOK], F32, tag="rstd_row", name="rstd_row")
                    nc.scalar.mul(rstd_row[:], ss_ps[:], 1.0 / DM)
                    mu2 = work.tile([1, TOK], F32, tag="rowtmp", name="mu2", bufs=1)
                    nc.vector.tensor_mul(mu2[:], mu_row[:], mu_row[:])
                    nc.vector.tensor_sub(rstd_row[:], rstd_row[:], mu2[:])
                    eps2 = wpool.tile([1, 1], F32, tag="eps2", name="eps2")
                    nc.vector.memset(eps2[:], 1e-5)
                    nc.scalar.activation(rstd_row[:], rstd_row[:], AF.Ln, bias=eps2[:, 0:1])
                    nc.scalar.activation(rstd_row[:], rstd_row[:], AF.Exp, scale=-0.5)
                    mu_rep = pln.tile([128, TOK], F32, tag="mu_rep", name="mu_rep")
                    rs_rep = pln.tile([128, TOK], F32, tag="rs_rep2", name="rs_rep2")
                    for fh in range(2):
                        fs = slice(fh * 512, (fh + 1) * 512)
                        nc.tensor.matmul(mu_rep[:, fs],ones1[:],mu_row[:, fs],
                                         start=True, stop=True)
                        nc.tensor.matmul(rs_rep[:, fs],ones1[:],rstd_row[:, fs],
                                         start=True, stop=True)
                    for m in range(MT):
                        nc.vector.tensor_sub(xn2[m][:], xn2[m][:], mu_rep[:])
                        nc.vector.tensor_mul(xn2[m][:], xn2[m][:], rs_rep[:])
                        nc.scalar.activation(xn2[m][:], xn2[m][:], AF.Identity,
                                             bias=lnb_t[:, m, 0:1],
                                             scale=lng_t[:, m, 0:1])
                        nc.sync.dma_start(outT_d[m * 128:(m + 1) * 128, :], xn2[m][:])

            if loop_k > 1:
                with tc.For_i(0, loop_k, 1):
                    body()
            else:
                body()

    nc.compile()
    _BUILD_CACHE[key] = nc
    return nc


# ======================================================================
# host entry
# ======================================================================

def _make_in_maps(inputs):
    x = np.asarray(inputs["x"], F32_np)
    fw = _prep_layer_weights(inputs["fm_in"], inputs["fm_convw"], inputs["fm_convb"],
                             inputs["fm_xproj"], inputs["fm_dtw"], inputs["fm_dtb"],
                             inputs["fm_Alog"], inputs["fm_D"], inputs["fm_out"],
                             inputs["fm_norm"])
    bw = _prep_layer_weights(inputs["bm_in"], inputs["bm_convw"], inputs["bm_convb"],
                             inputs["bm_xproj"], inputs["bm_dtw"], inputs["bm_dtb"],
                             inputs["bm_Alog"], inputs["bm_D"], inputs["bm_out"],
                             inputs["bm_norm"])
    sh = _prep_shared_weights(inputs["proj_w"], inputs["proj_b"],
                              inputs["ln_g"], inputs["ln_b"])
    base = {}
    for s, w in (("f", fw), ("b", bw)):
        for k, v in w.items():
            if k in ("convbn", "convb", "inw", "convd", "xpw", "dtw", "dtb", "A", "Dp",
                     "outw"):
                base[f"{s}_{k}"] = v
    base["projw"] = sh["projw"]
    base["projb"] = sh["projb"]
    base["lng"] = sh["lng"]
    base["lnb"] = sh["lnb"]

    in_maps = []
    for c in range(NCORES):
        xc = x[c * BL:(c + 1) * BL]                       # (BL, L, DM)
        xTc = np.ascontiguousarray(xc.reshape(TOK, DM).T).astype(BF16_np)  # (DM, TOK)
        m = dict(base)
        m["xT"] = xTc
        in_maps.append(m)
    return in_maps


def _unshard(results):
    outs = []
    for c in range(NCORES):
        oT = results[c]["outT"]                            # (DM, TOK)
        outs.append(np.ascontiguousarray(oT.T.reshape(BL, L, DM)))
    return np.concatenate(outs, axis=0).astype(F32_np)


def _cfg_for_inputs(inputs, base_cfg=None):
    """Enable the immediate-scale dA path when A rows are d-independent."""
    cfg = dict(CFG if base_cfg is None else base_cfg)
    parts = []
    for s, key in (("f", "fm_Alog"), ("b", "bm_Alog")):
        A = -np.exp(np.asarray(inputs[key], np.float64).astype(np.float64))
        if np.allclose(A, A[0:1, :], rtol=1e-6, atol=1e-9):
            a32 = A[0, :].astype(np.float32)
            parts.append(s + ":" + " ".join(repr(float(v)) for v in a32))
    if parts:
        cfg["ACONST"] = ";".join(parts)
    return cfg


def kernel(**inputs):
    from concourse import bass_utils
    nc = _build(loop_k=1, cfg=_cfg_for_inputs(inputs))
    in_maps = _make_in_maps(inputs)
    res = bass_utils.run_bass_kernel_spmd(nc, in_maps, core_ids=list(range(NCORES)))
    return _unshard(res.results)

